# revision 1
# baseline (speedup 1.0000x reference)
"""DeepseekV2-style MoE (64 experts, top-6 grouped sigmoid routing) on 8 TRN2
NeuronCores.

Expert-parallel: 8 experts per core (LPT-balanced placement, using a cheap
host-side peek at routing for placement/capacity ONLY -- the device recomputes
the full routing numerically). Shared experts sharded along their intermediate
dim. Per-core device pipeline:

  load+transpose x -> gate matmul -> sigmoid -> grouped top-k (DVE) ->
  per-expert rank via triangular matmul -> slot scatter (dma_scatter_add into
  an HBM index buffer) -> readback -> ap_gather token dispatch (SBUF) ->
  per-expert MLPs in fp32r -> routing-weighted dma_scatter_add combine into an
  out_partial pre-initialized with the shared-expert output -> ReduceScatter
  -> per-core [256, 1024] shard; host concatenates.
"""

import sys

sys.path.insert(0, "/opt/trn_rl_repo")

import numpy as np

from concourse import bacc, tile, library_config
import concourse.mybir as mybir
from concourse.bass_utils import run_bass_kernel_spmd

FP32 = mybir.dt.float32
FP32R = mybir.dt.float32r
BF16 = mybir.dt.bfloat16
I16 = mybir.dt.int16
ALU = mybir.AluOpType
AXL = mybir.AxisListType
ACTF = mybir.ActivationFunctionType

T = 2048
HID = 1024
E = 64
INTER = 704
K = 6
NG = 8
TG = 4
ISH = 1408
SCALE = 2.5
NC = 8
EPC = 8
NEG = -1.0e30

NTT = T // 128            # 16 token tiles
NH = HID // 128           # 8 hid chunks
IC_SIZES = [128, 128, 128, 128, 128, 64]
IC_OFFS = [0, 128, 256, 384, 512, 640]
GS = E // NG              # 8 experts / group
PAY_S = K * NTT           # 96


def _host_routing_counts(x, gate_w, score_bias):
    logits = x.astype(np.float64) @ gate_w.T.astype(np.float64)
    scores = 1.0 / (1.0 + np.exp(-logits))
    sc = scores + score_bias[None, :]
    gs = sc.reshape(T, NG, GS)
    top2 = np.sort(gs, axis=-1)[:, :, -2:].sum(-1)
    gidx = np.argsort(-top2, axis=-1)[:, :TG]
    gmask = np.zeros((T, NG), np.float64)
    np.put_along_axis(gmask, gidx, 1.0, axis=1)
    smask = np.repeat(gmask, GS, axis=1)
    masked = np.where(smask > 0, sc, -np.inf)
    ids = np.argsort(-masked, axis=-1)[:, :K]
    cnt = np.zeros(E, np.int64)
    for k in range(K):
        cnt += np.bincount(ids[:, k], minlength=E)
    return cnt


def _placement(caps):
    """Every core runs the identical capacity schedule, so total work is
    NC * sum(cap_sched) regardless of which core owns which expert. The
    only thing to minimize is sum_p max_c cap[bins[c][p]], which rank-based
    packing achieves: position p holds experts ranked [NC*p, NC*(p+1))."""
    order = np.argsort(-caps)
    bins = [[int(order[NC * p + c]) for p in range(EPC)] for c in range(NC)]
    cap_sched = [int(caps[order[NC * p]]) for p in range(EPC)]
    return bins, cap_sched


def _blocks(cap):
    sizes = [256] * (cap // 256)
    if cap % 256:
        sizes.append(128)
    return sizes


def build_graph(cap_sched, s_core, s_total, stage=99):
    """cap_sched: [EPC] per-position capacities (multiples of 128), identical
    on every core. s_core = sum(cap_sched). s_total = NC * s_core.
    stage: truncate pipeline for debugging (99 = full)."""
    nc = bacc.Bacc("TRN2", target_bir_lowering=False, debug=False,
                   num_devices=NC, num_swdge_queues=2)

    x_d = nc.dram_tensor("x", [T, HID], FP32, kind="ExternalInput")
    gwT_d = nc.dram_tensor("gwT", [NH, 128, E], FP32, kind="ExternalInput")
    bias_d = nc.dram_tensor("bias_b", [128, E], FP32, kind="ExternalInput")
    wg_d = nc.dram_tensor("wg", [EPC, HID, INTER], FP32R, kind="ExternalInput")
    wu_d = nc.dram_tensor("wu", [EPC, HID, INTER], FP32R, kind="ExternalInput")
    wd_d = nc.dram_tensor("wd", [EPC, INTER, HID], FP32R, kind="ExternalInput")
    shg_d = nc.dram_tensor("shg", [HID, ISH // NC], FP32R, kind="ExternalInput")
    shu_d = nc.dram_tensor("shu", [HID, ISH // NC], FP32R, kind="ExternalInput")
    shd_d = nc.dram_tensor("shd", [ISH // NC, HID], FP32R, kind="ExternalInput")
    ident_d = nc.dram_tensor("ident", [128, 128], FP32, kind="ExternalInput")
    ones_d = nc.dram_tensor("ones128", [128, 128], BF16, kind="ExternalInput")
    triu_d = nc.dram_tensor("triu128", [128, 128], BF16, kind="ExternalInput")
    base_d = nc.dram_tensor("base_b", [128, E], FP32, kind="ExternalInput")
    cap_d = nc.dram_tensor("cap_b", [128, E], FP32, kind="ExternalInput")
    tokpay_d = nc.dram_tensor("tokpay", [128, PAY_S], FP32,
                              kind="ExternalInput")
    out_d = nc.dram_tensor("out", [T // NC, HID], FP32, kind="ExternalOutput")

    ISH_C = ISH // NC                       # 176
    SH_IC = [(128, 0), (48, 128)]           # shared inter chunks

    rows_alloc = ((s_total + 128 + 1023) // 1024) * 1024

    with tile.TileContext(nc) as tc:
        with (
            tc.tile_pool(name="pers", bufs=1) as pers,
            tc.tile_pool(name="dram", bufs=1, space="DRAM") as dram,
        ):
            # constants
            ident = pers.tile([128, 128], FP32, tag="ident")
            nc.sync.dma_start(ident[:], ident_d[:])
            ones_sb = pers.tile([128, 128], BF16, tag="ones")
            nc.sync.dma_start(ones_sb[:], ones_d[:])
            triu_sb = pers.tile([128, 128], BF16, tag="triu")
            nc.sync.dma_start(triu_sb[:], triu_d[:])
            gw_sb = pers.tile([128, NH, E], FP32, tag="gw")
            nc.sync.dma_start(gw_sb[:], gwT_d.ap().transpose([1, 0, 2]))
            bias_sb = pers.tile([128, E], FP32, tag="bias")
            nc.sync.dma_start(bias_sb[:], bias_d[:])
            base_sb = pers.tile([128, E], FP32, tag="base")
            nc.sync.dma_start(base_sb[:], base_d[:])
            cap_sb = pers.tile([128, E], FP32, tag="cap")
            nc.sync.dma_start(cap_sb[:], cap_d[:])
            tokpay_sb = pers.tile([128, PAY_S], FP32, tag="tokpay")
            nc.sync.dma_start(tokpay_sb[:], tokpay_d[:])

            # routing results that outlive the routing scope
            tok_rep = pers.tile([128, s_core // 16], I16, tag="tokr")
            w_slots = pers.tile([128, s_core // 128], FP32, tag="wslots")

            # internal DRAM
            idx_buf = dram.tile([rows_alloc, E], FP32)
            out_part = dram.tile([T, HID], FP32)
            rs_out = dram.tile([T // NC, HID], FP32)

            # ---- 1. load + transpose x, fp32 gate logits on the fly ----
            scores = pers.tile([128, NTT, E], FP32, tag="scores")
            xtp = tc.alloc_tile_pool(name="xT", bufs=1, side="right")
            xT = [xtp.tile([128, T], FP32R, tag=f"xT{j}", name=f"xT{j}")
                  for j in range(NH)]
            psA = tc.alloc_tile_pool(name="psA", bufs=2, space="PSUM")
            iop = tc.alloc_tile_pool(name="iop", bufs=2)
            for i in range(NTT):
                xt = iop.tile([128, HID], FP32, tag="xin")
                nc.sync.dma_start(xt[:], x_d[128 * i:128 * (i + 1), :])
                pl = psA.tile([128, E], FP32, tag="plog")
                for j in range(NH):
                    pt = psA.tile([128, 128], FP32, tag="ptr")
                    nc.tensor.transpose(pt[:], xt[:, 128 * j:128 * (j + 1)],
                                        ident[:])
                    nc.vector.tensor_copy(xT[j][:, 128 * i:128 * (i + 1)],
                                          pt[:])
                    xtg = iop.tile([128, 128], FP32, tag="xtg", bufs=3)
                    nc.vector.tensor_copy(xtg[:], pt[:])
                    nc.tensor.matmul(pl[:], xtg[:], gw_sb[:, j, :],
                                     start=(j == 0), stop=(j == NH - 1))
                nc.scalar.activation(scores[:, i, :], pl[:], ACTF.Sigmoid)
            iop.release()

            if stage == 10:
                nc.sync.dma_start(out_d[0:128, :], xT[0][:, 0:1024].bitcast(FP32))
                nc.sync.dma_start(out_d[128:256, :], xT[0][:, 1024:2048].bitcast(FP32))
            # ---- 2..5: routing ----
            if stage >= 20:
                rp = tc.alloc_tile_pool(name="rout", bufs=1)
                self_routing(nc, tc, rp, psA, scores, bias_sb,
                             base_sb, cap_sb, tokpay_sb, ones_sb,
                             triu_sb, idx_buf, tok_rep, w_slots,
                             s_core, s_total, rows_alloc, stage, out_d)
                rp.release()
            psA.release()

            if stage >= 50:
                # ---- 6. shared experts -> out_part init ----
                shwp = tc.alloc_tile_pool(name="shw", bufs=1)
                shhp = tc.alloc_tile_pool(name="shh", bufs=3)
                stpB = tc.alloc_tile_pool(name="stB", bufs=2)
                psB = tc.alloc_tile_pool(name="psB", bufs=2, space="PSUM")
                psBy = tc.alloc_tile_pool(name="psBy", bufs=2, space="PSUM")
                shg_sb = shwp.tile([128, NH, ISH_C], FP32R, tag="shg")
                nc.sync.dma_start(
                    shg_sb[:], shg_d.ap().rearrange("(j p) i -> p j i", p=128))
                shu_sb = shwp.tile([128, NH, ISH_C], FP32R, tag="shu")
                nc.sync.dma_start(
                    shu_sb[:], shu_d.ap().rearrange("(j p) i -> p j i", p=128))
                shd_sb = shwp.tile([128, 2, HID], FP32R, tag="shd")
                nc.sync.dma_start(shd_sb[0:128, 0, :], shd_d[0:128, :])
                nc.sync.dma_start(shd_sb[0:48, 1, :], shd_d[128:176, :])

                for tb in range(4):
                    hs_t = []
                    for ci, (csz, coff) in enumerate(SH_IC):
                        pg = psB.tile([128, 512], FP32, tag="shpg")
                        pu = psB.tile([128, 512], FP32, tag="shpu")
                        for j in range(NH):
                            rhs = xT[j][:, 512 * tb:512 * (tb + 1)] 
                            nc.tensor.matmul(
                                pg[0:csz, :],
                                shg_sb[:, j, coff:coff + csz],
                                rhs, start=(j == 0), stop=(j == NH - 1))
                            nc.tensor.matmul(
                                pu[0:csz, :],
                                shu_sb[:, j, coff:coff + csz],
                                rhs, start=(j == 0), stop=(j == NH - 1))
                        hst = shhp.tile([128, 512], FP32R, tag="hsh")
                        nc.scalar.activation(hst[0:csz, :], pg[0:csz, :],
                                             ACTF.Sigmoid)
                        nc.vector.tensor_tensor(hst[0:csz, :], hst[0:csz, :],
                                                pg[0:csz, :], ALU.mult)
                        nc.vector.tensor_tensor(hst[0:csz, :], hst[0:csz, :],
                                                pu[0:csz, :], ALU.mult)
                        hs_t.append((hst, csz))
                    for st in range(4):
                        for nh2 in range(2):
                            py = psBy.tile([128, 512], FP32, tag="shpy")
                            for ci, ((hst, csz), _) in enumerate(
                                    zip(hs_t, SH_IC)):
                                nc.tensor.matmul(
                                    py[:],
                                    hst[0:csz, 128 * st:128 * (st + 1)]
                                    ,
                                    shd_sb[0:csz, ci, 512 * nh2:512 * (nh2 + 1)]
                                    ,
                                    start=(ci == 0), stop=(ci == 1))
                            ot = stpB.tile([128, 512], FP32, tag="osh")
                            nc.vector.tensor_copy(ot[:], py[:])
                            r0 = 512 * tb + 128 * st
                            nc.sync.dma_start(
                                out_part[r0:r0 + 128,
                                         512 * nh2:512 * (nh2 + 1)], ot[:])
                stpB.release()
                shhp.release()
                shwp.release()
                psBy.release()
                psB.release()


                if stage == 50:
                    sb5 = tc.alloc_tile_pool(name="sb5", bufs=2)
                    for i in range(2):
                        t5 = sb5.tile([128, HID], FP32, tag="t5")
                        nc.sync.dma_start(
                            t5[:], out_part[128 * i:128 * (i + 1), :])
                        nc.sync.dma_start(
                            out_d[128 * i:128 * (i + 1), :], t5[:])
                    sb5.release()
            if stage >= 60:
                # ---- 7. gather X^T via dma_gather + PE transpose ----
                xhp = tc.alloc_tile_pool(name="xhat", bufs=1)
                xhat = [xhp.tile([128, s_core], FP32R, tag=f"xh{j}",
                                 name=f"xh{j}") for j in range(NH)]
                xtp.release()
                psG = tc.alloc_tile_pool(name="psG", bufs=3, space="PSUM")
                iog = tc.alloc_tile_pool(name="iog", bufs=3)
                gsems = []
                grows = []
                for chk in range(s_core // 128):
                    grow = iog.tile([128, 1, HID], FP32, tag="grow")
                    with tc.tile_critical():
                        if gsems:
                            nc.gpsimd.wait_ge(gsems[-1], 16)
                            # anchor: readers of the landed chunk must
                            # schedule after this section
                            nc.gpsimd.tensor_copy(
                                grows[-1][0:1, 0, 0:1],
                                grows[-1][0:1, 0, 0:1])
                        gs_ = nc.alloc_semaphore(f"gx{chk}")
                        gp_ = nc.alloc_semaphore(f"gxp{chk}")
                        nc.gpsimd.dma_gather(
                            grow[:], x_d[:],
                            tok_rep[:, 8 * chk:8 * (chk + 1)],
                            128, 128, HID, elem_step=HID,
                            prepare_only=True, sem=gs_,
                            queue_num=1).then_inc(gp_, 1)
                        nc.gpsimd.wait_ge(gp_, 1)
                        nc.gpsimd.trigger_dma(1, queue_num=1)
                        gsems.append(gs_)
                        grows.append(grow)
                    if chk >= 1:
                        gprev = grows[chk - 1]
                        for j in range(NH):
                            ptg = psG.tile([128, 128], FP32, tag="ptg")
                            nc.tensor.transpose(
                                ptg[:], gprev[:, 0, 128 * j:128 * (j + 1)],
                                ident[:])
                            nc.vector.tensor_copy(
                                xhat[j][:, 128 * (chk - 1):128 * chk],
                                ptg[:])
                with tc.tile_critical():
                    nc.gpsimd.wait_ge(gsems[-1], 16)
                    nc.gpsimd.tensor_copy(grows[-1][0:1, 0, 0:1],
                                          grows[-1][0:1, 0, 0:1])
                lastc = s_core // 128 - 1
                for j in range(NH):
                    ptg = psG.tile([128, 128], FP32, tag="ptg")
                    nc.tensor.transpose(
                        ptg[:], grows[-1][:, 0, 128 * j:128 * (j + 1)],
                        ident[:])
                    nc.vector.tensor_copy(
                        xhat[j][:, 128 * lastc:128 * (lastc + 1)], ptg[:])
                psG.release()
                iog.release()


                if stage == 60:
                    nc.sync.dma_start(out_d[0:128, :], xhat[0][:, 0:1024].bitcast(FP32))
                    nc.sync.dma_start(out_d[128:256, :],
                                      xhat[0][:, 1024:2048].bitcast(FP32))
            else:
                xtp.release()
            if stage >= 70:
                # ---- 8. expert loop ----
                wp = tc.alloc_tile_pool(name="wp", bufs=2)
                wdp = tc.alloc_tile_pool(name="wdp", bufs=1)
                hp = tc.alloc_tile_pool(name="hp", bufs=1)
                stpE = tc.alloc_tile_pool(name="stE", bufs=2)
                psE = tc.alloc_tile_pool(name="psE", bufs=2, space="PSUM")
                psEy = tc.alloc_tile_pool(name="psEy", bufs=2, space="PSUM")
                lbase = 0
                ysems = []
                for e in range(EPC):
                    cap = cap_sched[e]
                    sizes = _blocks(cap)
                    He = [hp.tile([128, cap], FP32R, tag=f"he{ci}",
                                  name=f"he{ci}_{e}") for ci in range(6)]
                    for ci, (csz, coff) in enumerate(zip(IC_SIZES, IC_OFFS)):
                        wgic = wp.tile([128, NH, 128], FP32R, tag="wgic")
                        wuic = wp.tile([128, NH, 128], FP32R, tag="wuic")
                        nc.sync.dma_start(
                            wgic[:, :, 0:csz],
                            wg_d[e].rearrange("(j p) i -> p j i",
                                              p=128)[:, :, coff:coff + csz])
                        nc.sync.dma_start(
                            wuic[:, :, 0:csz],
                            wu_d[e].rearrange("(j p) i -> p j i",
                                              p=128)[:, :, coff:coff + csz])
                        boff = 0
                        for bsz in sizes:
                            bl = lbase + boff
                            pg = psE.tile([128, 256], FP32, tag="epg")
                            pu = psE.tile([128, 256], FP32, tag="epu")
                            for kk in range(NH):
                                rhs = xhat[kk][:, bl:bl + bsz]
                                nc.tensor.matmul(
                                    pg[0:csz, 0:bsz],
                                    wgic[:, kk, 0:csz],
                                    rhs, start=(kk == 0), stop=(kk == NH - 1))
                                nc.tensor.matmul(
                                    pu[0:csz, 0:bsz],
                                    wuic[:, kk, 0:csz],
                                    rhs, start=(kk == 0), stop=(kk == NH - 1))
                            nc.scalar.activation(
                                He[ci][0:csz, boff:boff + bsz],
                                pg[0:csz, 0:bsz], ACTF.Sigmoid)
                            nc.vector.tensor_tensor(
                                He[ci][0:csz, boff:boff + bsz],
                                He[ci][0:csz, boff:boff + bsz],
                                pg[0:csz, 0:bsz], ALU.mult)
                            nc.vector.tensor_tensor(
                                He[ci][0:csz, boff:boff + bsz],
                                He[ci][0:csz, boff:boff + bsz],
                                pu[0:csz, 0:bsz], ALU.mult)
                            boff += bsz

                    wd_sb = wdp.tile([128, 6, HID], FP32R, tag="wd",
                                     name=f"wd_{e}")
                    for ci, (csz, coff) in enumerate(zip(IC_SIZES, IC_OFFS)):
                        nc.sync.dma_start(wd_sb[0:csz, ci, :],
                                          wd_d[e, coff:coff + csz, :])
                    boff = 0
                    for bsz in sizes:
                        bl = lbase + boff
                        stg = stpE.tile([128, 2, HID], FP32, tag="ystg", bufs=3)
                        for sc_i in range(bsz // 128):
                            so = boff + 128 * sc_i
                            for nh2 in range(2):
                                py = psEy.tile([128, 512], FP32, tag="epy")
                                for ci, csz in enumerate(IC_SIZES):
                                    nc.tensor.matmul(
                                        py[:],
                                        He[ci][0:csz, so:so + 128]
                                        ,
                                        wd_sb[0:csz, ci,
                                              512 * nh2:512 * (nh2 + 1)]
                                        ,
                                        start=(ci == 0), stop=(ci == 5))
                                nc.vector.tensor_scalar_mul(
                                    stg[:, sc_i, 512 * nh2:512 * (nh2 + 1)],
                                    py[:],
                                    w_slots[:, (bl + 128 * sc_i) // 128:
                                            (bl + 128 * sc_i) // 128 + 1])
                        ysem = nc.alloc_semaphore(f"swdge_y{e}_{boff}")
                        nc.gpsimd.dma_scatter_add(
                            out_part[:], stg[:, 0:bsz // 128, :],
                            tok_rep[:, bl // 16:(bl + bsz) // 16],
                            bsz, bsz, HID,
                            prepare_only=True, sem=ysem)
                        nc.gpsimd.trigger_dma(count=None)
                        boff += bsz
                    lbase += cap
                stpE.release()
                hp.release()
                wdp.release()
                wp.release()
                psEy.release()
                psE.release()


                if stage == 70:
                    sb7 = tc.alloc_tile_pool(name="sb7", bufs=2)
                    for i in range(2):
                        t7 = sb7.tile([128, HID], FP32, tag="t7")
                        nc.sync.dma_start(
                            t7[:], out_part[128 * i:128 * (i + 1), :])
                        nc.sync.dma_start(
                            out_d[128 * i:128 * (i + 1), :], t7[:])
                    sb7.release()
            if stage >= 60:
                xhp.release()
            if stage >= 70:
                with tc.tile_critical():
                    for ys in ysems:
                        nc.gpsimd.wait_ge(ys, 16)
                    d2sem = nc.alloc_semaphore("y_drain")
                    nc.gpsimd.dma_start(
                        tokpay_sb[0:1, 0:1],
                        out_part[0:1, 0:1]).then_inc(d2sem, 16)
                    nc.gpsimd.wait_ge(d2sem, 16)
            # ---- 9. reduce-scatter + output ----
            if stage < 99:
                pass
            else:
                nc.gpsimd.collective_compute(
                "ReduceScatter", ALU.add,
                    replica_groups=[list(range(NC))],
                    ins=[out_part.opt()], outs=[rs_out.opt()])
                iop2 = tc.alloc_tile_pool(name="iop2", bufs=2)
                for i in range(T // NC // 128):
                    ot2 = iop2.tile([128, HID], FP32, tag="outld")
                    nc.sync.dma_start(ot2[:],
                                      rs_out[128 * i:128 * (i + 1), :])
                    nc.sync.dma_start(out_d[128 * i:128 * (i + 1), :],
                                      ot2[:])
                iop2.release()

    nc.compile()
    return nc


def self_routing(nc, tc, rp, psA, scores, bias_sb, base_sb, cap_sb,
                 tokpay_sb, ones_sb, triu_sb, idx_buf, tok_rep, w_slots,
                 s_core, s_total, rows_alloc, stage=99, out_d=None):
    """Gate matmul, sigmoid, grouped top-k, rank matmul, slot scatter,
    readback. Writes tok_rep (int16 token per slot, replicated x8) and
    w_slots (scaled routing weight per slot)."""
    if stage == 20:
        nc.sync.dma_start(out_d[0:128, :],
                          scores[:].rearrange("p t e -> p (t e)"))
        return
    sc_b = rp.tile([128, NTT, E], FP32, tag="scb")
    nc.vector.tensor_tensor(
        sc_b[:], scores[:],
        bias_sb[:].unsqueeze(1).to_broadcast([128, NTT, E]), ALU.add)

    scg = sc_b[:].rearrange("p t (g s) -> p t g s", g=NG, s=GS)
    m1 = rp.tile([128, NTT, NG], FP32, tag="m1")
    nc.vector.tensor_reduce(m1[:], scg, AXL.X, ALU.max)
    oh1 = rp.tile([128, NTT, NG, GS], FP32, tag="oh1")
    nc.vector.tensor_tensor(
        oh1[:], scg,
        m1[:].unsqueeze(3).to_broadcast([128, NTT, NG, GS]), ALU.is_ge)
    msk2 = rp.tile([128, NTT, NG, GS], FP32, tag="msk2")
    nc.vector.scalar_tensor_tensor(msk2[:], oh1[:], NEG, scg,
                                   ALU.mult, ALU.add)
    m2 = rp.tile([128, NTT, NG], FP32, tag="m2")
    nc.vector.tensor_reduce(m2[:], msk2[:], AXL.X, ALU.max)
    gsc = rp.tile([128, NTT, NG], FP32, tag="gsc")
    nc.vector.tensor_tensor(gsc[:], m1[:], m2[:], ALU.add)

    gmask = rp.tile([128, NTT, NG], FP32, tag="gmask")
    nc.vector.memset(gmask[:], 0.0)
    for g in range(TG):
        gm = rp.tile([128, NTT, 1], FP32, tag="gm")
        nc.vector.tensor_reduce(gm[:], gsc[:], AXL.X, ALU.max)
        ohg = rp.tile([128, NTT, NG], FP32, tag="ohg")
        nc.vector.tensor_tensor(ohg[:], gsc[:],
                                gm[:].to_broadcast([128, NTT, NG]),
                                ALU.is_ge)
        nc.vector.tensor_tensor(gmask[:], gmask[:], ohg[:], ALU.add)
        nc.vector.scalar_tensor_tensor(gsc[:], ohg[:], NEG, gsc[:],
                                       ALU.mult, ALU.add)

    # masked = sc where group selected else -1e30, computed exactly:
    # sel: sc*1 + (1e30 - 1e30) = sc ; unsel: 0 + (0 - 1e30)
    masked = rp.tile([128, NTT, E], FP32, tag="masked")
    mview = masked[:].rearrange("p t (g s) -> p t g s", g=NG, s=GS)
    nc.vector.tensor_tensor(
        mview, scg,
        gmask[:].unsqueeze(3).to_broadcast([128, NTT, NG, GS]), ALU.mult)
    gb = rp.tile([128, NTT, NG], FP32, tag="gb")
    nc.vector.tensor_scalar(gb[:], gmask[:], 1.0e30, -1.0e30,
                            ALU.mult, ALU.add)
    nc.vector.tensor_tensor(
        mview, mview,
        gb[:].unsqueeze(3).to_broadcast([128, NTT, NG, GS]), ALU.add)

    ohs = []
    msel = rp.tile([128, NTT, E], FP32, tag="msel")
    nc.vector.memset(msel[:], 0.0)
    for k in range(K):
        mk = rp.tile([128, NTT, 1], FP32, tag=f"mk{k}")
        nc.vector.tensor_reduce(mk[:], masked[:], AXL.X, ALU.max)
        ohk = rp.tile([128, NTT, E], FP32, tag=f"ohk{k}")
        nc.vector.tensor_tensor(ohk[:], masked[:],
                                mk[:].to_broadcast([128, NTT, E]),
                                ALU.is_ge)
        ohs.append(ohk)
        nc.vector.scalar_tensor_tensor(masked[:], ohk[:], NEG, masked[:],
                                       ALU.mult, ALU.add)
        nc.vector.tensor_tensor(msel[:], msel[:], ohk[:], ALU.add)

    # denom = sum(msel * scores) ; rden = 1/denom
    wr = rp.tile([128, NTT, E], FP32, tag="wr")
    nc.vector.tensor_tensor(wr[:], msel[:], scores[:], ALU.mult)
    denom = rp.tile([128, NTT], FP32, tag="denom")
    nc.vector.tensor_reduce(denom[:], wr[:], AXL.X, ALU.add)
    rden = rp.tile([128, NTT], FP32, tag="rden")
    nc.vector.reciprocal(rden[:], denom[:])

    # rank matmul (bf16 0/1 inputs, fp32 psum)
    msel_bf = rp.tile([128, NTT, E], BF16, tag="mselbf")
    nc.vector.tensor_copy(msel_bf[:], msel[:])
    R = rp.tile([128, NTT, E], FP32, tag="R")
    for i in range(NTT):
        pr = psA.tile([128, E], FP32, tag="pr")
        n_mm = i + 1
        for mi in range(n_mm):
            lhsT = ones_sb[:] if mi < i else triu_sb[:]
            nc.tensor.matmul(pr[:], lhsT, msel_bf[:, mi, :],
                             start=(mi == 0), stop=(mi == n_mm - 1))
        nc.vector.tensor_copy(R[:, i, :], pr[:])

    BR = rp.tile([128, NTT, E], FP32, tag="BR")
    nc.vector.tensor_tensor(
        BR[:], R[:], base_sb[:].unsqueeze(1).to_broadcast([128, NTT, E]),
        ALU.add)
    OV = rp.tile([128, NTT, E], FP32, tag="OV")
    nc.vector.tensor_tensor(
        OV[:], R[:], cap_sb[:].unsqueeze(1).to_broadcast([128, NTT, E]),
        ALU.is_ge)

    dest = rp.tile([128, K, NTT], FP32, tag="dest")
    wpay = rp.tile([128, K, NTT], FP32, tag="wpay")
    tmp = rp.tile([128, NTT, E], FP32, tag="tmpke")
    trash = rp.tile([128, NTT], FP32, tag="trash")
    nc.vector.memset(trash[:], float(s_total))
    for k in range(K):
        nc.vector.tensor_tensor(tmp[:], ohs[k][:], BR[:], ALU.mult)
        dk = rp.tile([128, NTT], FP32, tag="dk")
        nc.vector.tensor_reduce(dk[:], tmp[:], AXL.X, ALU.add)
        nc.vector.tensor_tensor(tmp[:], ohs[k][:], OV[:], ALU.mult)
        ovk = rp.tile([128, NTT], FP32, tag="ovk")
        nc.vector.tensor_reduce(ovk[:], tmp[:], AXL.X, ALU.add)
        ovk_u8 = rp.tile([128, NTT], mybir.dt.uint8, tag="ovku")
        nc.vector.tensor_copy(ovk_u8[:], ovk[:])
        nc.vector.copy_predicated(dk[:], ovk_u8[:], trash[:])
        nc.vector.tensor_copy(dest[:, k, :], dk[:])
        nc.vector.tensor_tensor(tmp[:], ohs[k][:], scores[:], ALU.mult)
        wk = rp.tile([128, NTT], FP32, tag="wk")
        nc.vector.tensor_reduce(wk[:], tmp[:], AXL.X, ALU.add)
        nc.vector.tensor_tensor(wk[:], wk[:], rden[:], ALU.mult)
        nc.vector.tensor_scalar_mul(wpay[:, k, :], wk[:], SCALE)

    if stage == 30:
        nc.sync.dma_start(
            out_d[0:128, 0:PAY_S],
            dest[:].rearrange("p k i -> p (k i)"))
        nc.sync.dma_start(
            out_d[0:128, 128:128 + PAY_S],
            wpay[:].rearrange("p k i -> p (k i)"))
        nc.sync.dma_start(out_d[128:256, :],
                          R[:].rearrange("p t e -> p (t e)"))
        return
    dest_i16 = rp.tile([128, K, NTT], I16, tag="desti")
    nc.vector.tensor_copy(dest_i16[:], dest[:])
    idxs_disp = rp.tile([128, K * T // 16], I16, tag="idxd")
    idv = idxs_disp[:].rearrange("p (k i b) -> p k i b", k=K, i=NTT, b=8)
    dsp = dest_i16[:].rearrange("(b q) k i -> b q k i", b=8)
    for b in range(8):
        nc.sync.dma_start(idv[0:16, :, :, b], dsp[b])
    # ucode Q7 cores each read their own 16-partition block: replicate x8
    for b in range(1, 8):
        nc.sync.dma_start(idxs_disp[16 * b:16 * (b + 1), :],
                          idxs_disp[0:16, :])

    pay = rp.tile([128, PAY_S, E], FP32, tag="pay")
    nc.vector.memset(pay[:], 0.0)
    nc.vector.tensor_copy(pay[:, :, 0:1], tokpay_sb[:].unsqueeze(2))
    nc.vector.tensor_copy(
        pay[:, :, 1:2],
        wpay[:].rearrange("p k i -> p (k i)").unsqueeze(2))

    # zero idx_buf
    zt = rp.tile([128, 512], FP32, tag="zt")
    nc.vector.memset(zt[:], 0.0)
    ztot = rows_alloc * E
    zchunk = 128 * 512
    ib_flat = idx_buf[:].rearrange("a b -> (a b)")
    for z in range(ztot // zchunk):
        nc.sync.dma_start(
            ib_flat[z * zchunk:(z + 1) * zchunk]
            .rearrange("(a b) -> a b", b=512), zt[:])

    if stage == 35:
        # dump idxs_disp (cast to f32) and payload cols
        idf = rp.tile([16, K * T // 16], FP32, tag="idf")
        nc.vector.tensor_copy(idf[:], idxs_disp[0:16, :])
        nc.sync.dma_start(out_d[0:16, 0:768], idf[:])
        nc.sync.dma_start(out_d[128:256, 0:PAY_S],
                          pay[:, :, 0].rearrange("p s -> p s"))
        nc.sync.dma_start(out_d[128:256, 128:128 + PAY_S], pay[:, :, 1])
        return
    pay_sems = [nc.alloc_semaphore(f"swdge_pay{ch}") for ch in range(K)]
    pp_sems = [nc.alloc_semaphore(f"swdge_pp{ch}") for ch in range(K)]
    for ch in range(K):
        with tc.tile_critical():
            if ch >= 2:
                nc.gpsimd.wait_ge(pay_sems[ch - 2], 16)
            nc.gpsimd.dma_scatter_add(
                idx_buf[:], pay[:, NTT * ch:NTT * (ch + 1), :],
                idxs_disp[:, 128 * ch:128 * (ch + 1)],
                T, T, E, prepare_only=True,
                sem=pay_sems[ch]).then_inc(pp_sems[ch], 1)
            nc.gpsimd.wait_ge(pp_sems[ch], 1)
            nc.gpsimd.trigger_dma(1)

    if stage == 36:
        sb36 = tc.alloc_tile_pool(name="sb36", bufs=2)
        for i in range(2):
            t3 = sb36.tile([128, E], FP32, tag="t3")
            nc.sync.dma_start(t3[:], idx_buf[128 * i:128 * (i + 1), :])
            nc.sync.dma_start(out_d[128 * i:128 * (i + 1), 0:E], t3[:])
        sb36.release()
        return

    # readback (explicitly gated on all payload-scatter completions)
    tok_f32 = rp.tile([16, s_core // 16], FP32, tag="tokf")
    with tc.tile_critical():
        for ps_ in pay_sems:
            nc.gpsimd.wait_ge(ps_, 16)
        rbsem = nc.alloc_semaphore("rb")
        nc.gpsimd.dma_start(
            tok_f32[:],
            idx_buf[:].transpose([1, 0])[0, 0:s_core]
            .rearrange("(s q) -> q s", q=16)).then_inc(rbsem, 16)
        nc.gpsimd.dma_start(
            w_slots[:],
            idx_buf[:].transpose([1, 0])[1, 0:s_core]
            .rearrange("(c p) -> p c", p=128)).then_inc(rbsem, 16)
        nc.gpsimd.wait_ge(rbsem, 32)
    tok_i16 = rp.tile([16, s_core // 16], I16, tag="toki")
    nc.vector.tensor_copy(tok_i16[:], tok_f32[:])
    for b in range(8):
        nc.sync.dma_start(tok_rep[16 * b:16 * (b + 1), :], tok_i16[:])
    if stage == 40:
        nc.sync.dma_start(out_d[0:16, 0:s_core // 16], tok_f32[:])
        nc.sync.dma_start(out_d[128:256, 0:s_core // 128], w_slots[:])
        return


def prepare(stage=99, **inputs):
    import ml_dtypes

    x = np.ascontiguousarray(inputs["hidden_states"], dtype=np.float32)
    gate_w = np.ascontiguousarray(inputs["gate_w"], dtype=np.float32)
    score_bias = np.ascontiguousarray(inputs["score_bias"], dtype=np.float32)

    cnt = _host_routing_counts(x, gate_w, score_bias)
    caps = ((cnt + 16 + 127) // 128 * 128).astype(np.int64)
    bins, cap_sched = _placement(caps)
    s_core = int(sum(cap_sched))
    s_total = NC * s_core

    pos_off = np.concatenate([[0], np.cumsum(cap_sched)[:-1]])
    owner = np.zeros(E, np.int64)
    opos = np.zeros(E, np.int64)
    for c in range(NC):
        for p, e in enumerate(bins[c]):
            owner[e] = c
            opos[e] = p

    ident = np.eye(128, dtype=np.float32)
    ones128 = np.ones((128, 128), dtype=ml_dtypes.bfloat16)
    triu128 = np.triu(np.ones((128, 128), np.float32), 1).astype(
        ml_dtypes.bfloat16)
    tokpay = np.zeros((128, PAY_S), np.float32)
    for k in range(K):
        for i in range(NTT):
            tokpay[:, k * NTT + i] = np.arange(128) + 128 * i
    cap_row = np.array([cap_sched[opos[e]] for e in range(E)], np.float32)

    ish_c = ISH // NC
    in_maps = []
    for c in range(NC):
        base = np.zeros(E, np.float32)
        for e in range(E):
            base[e] = pos_off[opos[e]] + s_core * int((owner[e] - c) % NC)
        perm = bins[c]
        in_maps.append({
            "x": x,
            "gwT": np.ascontiguousarray(gate_w.T.reshape(NH, 128, E)),
            "bias_b": np.ascontiguousarray(
                np.tile(score_bias, (128, 1))),
            "wg": np.ascontiguousarray(
                np.asarray(inputs["w_gate"], np.float32)[perm]),
            "wu": np.ascontiguousarray(
                np.asarray(inputs["w_up"], np.float32)[perm]),
            "wd": np.ascontiguousarray(
                np.asarray(inputs["w_down"], np.float32)[perm]),
            "shg": np.ascontiguousarray(
                np.asarray(inputs["sh_gate"],
                           np.float32)[:, c * ish_c:(c + 1) * ish_c]),
            "shu": np.ascontiguousarray(
                np.asarray(inputs["sh_up"],
                           np.float32)[:, c * ish_c:(c + 1) * ish_c]),
            "shd": np.ascontiguousarray(
                np.asarray(inputs["sh_down"],
                           np.float32)[c * ish_c:(c + 1) * ish_c, :]),
            "ident": ident,
            "ones128": ones128,
            "triu128": triu128,
            "base_b": np.ascontiguousarray(np.tile(base, (128, 1))),
            "cap_b": np.ascontiguousarray(np.tile(cap_row, (128, 1))),
            "tokpay": tokpay,
        })

    nc = build_graph(cap_sched, s_core, s_total, stage=stage)
    return nc, in_maps


def kernel(stage=99, **inputs):
    nc, in_maps = prepare(stage=stage, **inputs)
    res = run_bass_kernel_spmd(nc, in_maps, core_ids=list(range(NC)))
    out = np.concatenate([res.results[c]["out"] for c in range(NC)], axis=0)
    return np.ascontiguousarray(out, dtype=np.float32)


if __name__ == "__main__":
    import reference
    inp = {k: np.asarray(v) for k, v in reference.setup_inputs().items()}
    out = kernel(**inp)
    print("out", out.shape, out.dtype)

